# revision 35
# baseline (speedup 1.0000x reference)
"""Trainium2 Bass kernel for nn_CDRsAttention (sparse multi-head attention
with padding mask + CDR key mask on the first 2 heads).

Sharding: 8 cores = 4 samples (B) x 2 head-parity groups. Core (b, p)
computes heads [p, p+2, p+4, p+6] of sample b (exactly one CDR head each,
so all cores do identical work), producing a partial output
ctx_heads @ Wo_rows; the host sums the two parity partials + bo_eff.

Host-side prep (pure numpy, cheap):
  - two key gathers per sample: region R = all keys with mask==1 (regular
    heads attend these), region C = the CDR-valid subset again (the CDR
    head attends only these). Separate regions cost duplicate k/v
    projection work but minimize padded exp slots, which is what the
    ScalarE exp roofline is made of. Pad slots have k=v=0 and a 0 in the
    ones-row, so they drop out of softmax numerator and denominator.
  - bias algebra: bk is softmax-invariant (dropped), bv shifts ctx by a
    constant per head (folded into bo on host), bq contributes bq.k per
    key (applied through the exp bias operand only when nonzero; zero
    here). Only the v projection keeps an augmented ones-row, feeding a
    selector column so ctx row 64 is the softmax denominator.

Device per core (bf16 data, f32 accumulation):
  warmup matmuls spin the PE clock up during the initial DMAs; k/q/v
  projections are emitted just-in-time inside the attention blocks' PE
  slack. Per (query-chunk, head-pair) block: packed score matmuls (two
  64-row PE groups) fill S [128,1024] slots pairing the CDR head's
  ktiles with regular ktiles (and doubling up leftover regular ktiles),
  so nearly every exp ACTIVATE on ScalarE - the binding engine at
  T x sum_h(keys_h) elements, 1/lane/cycle - is full width. ctx^T
  accumulates in PSUM with the denominator in row 64; normalize =
  DVE copy (frees PSUM) + reciprocal on a [64,8] reshape (DVE reciprocal
  is ~6.5 cyc/free-elem, so keep the free dim tiny) + a DRAM-row
  partition-broadcast DMA + DVE multiply. Output projection chunks drain
  into remaining PE slack; per-128-row bf16 output DMAs overlap the
  tail.
"""
import os
from contextlib import ExitStack

import ml_dtypes
import numpy as np

import concourse.bass as bass
import concourse.mybir as mybir
import concourse.tile as tile
from concourse import bacc
from concourse.bass_utils import run_bass_kernel_spmd

B, T, C, H, D = 4, 2048, 512, 8, 64
F32 = mybir.dt.float32
BF16 = mybir.dt.bfloat16
EXP_SHIFT = -3.0  # exp(s - 3): softmax-invariant (keeps P modest)
EXP_SCALE = 1.0 / 8.0

_PROGRAM_CACHE: dict = {}
LAST_RESULTS = None  # BassKernelResults of the most recent kernel() call


def _chunks(total, step):
    return [(i, min(step, total - i)) for i in range(0, total, step)]


def _build_program(NKR, NCDR, use_qbias):
    NK = NKR + NCDR  # total key slots: [regular | cdr]
    NKT = NK // 128
    NKTR = NKR // 128
    NKTC = NCDR // 128

    nc = bacc.Bacc("TRN2", target_bir_lowering=False, debug=False, num_devices=8)
    # free layouts: xT/xkT/wq/wk/wv are (ci, n): contraction c = ci*128+ki
    xT_d = nc.dram_tensor("xT", [128, 4 * T], BF16, kind="ExternalInput").ap()
    xkT_d = nc.dram_tensor("xkT", [128, 4 * NK], BF16, kind="ExternalInput").ap()
    ones_d = nc.dram_tensor("ones", [1, NK], BF16, kind="ExternalInput").ap()
    wq_d = nc.dram_tensor("Wq", [128, 4 * 256], BF16, kind="ExternalInput").ap()
    wk_d = nc.dram_tensor("Wk", [128, 4 * 256], BF16, kind="ExternalInput").ap()
    wv_d = nc.dram_tensor("Wv", [128, 4 * 260], BF16, kind="ExternalInput").ap()
    wva_d = nc.dram_tensor("Wva", [1, 260], BF16, kind="ExternalInput").ap()
    wo_d = nc.dram_tensor("Wo", [256, 512], BF16, kind="ExternalInput").ap()
    bq_d = (nc.dram_tensor("bq", [128, 2], BF16, kind="ExternalInput").ap()
            if use_qbias else None)
    out_d = nc.dram_tensor("out", [T, 512], BF16, kind="ExternalOutput").ap()
    scr_d = nc.dram_tensor("scr", [16, 512], F32, kind="Internal").ap()

    with tile.TileContext(nc) as tc:
        with ExitStack() as ctx:
            _body(ctx, tc, xT_d, xkT_d, ones_d, wq_d, wk_d, wv_d, wva_d,
                  wo_d, bq_d, out_d, scr_d, NK, NKT, NKTR, NKTC, use_qbias)
    nc.compile()
    return nc


def _body(ctx, tc, xT_d, xkT_d, ones_d, wq_d, wk_d, wv_d, wva_d, wo_d, bq_d,
          out_d, scr_d, NK, NKT, NKTR, NKTC, use_qbias):
    nc = tc.nc
    Exp = mybir.ActivationFunctionType.Exp

    xpool = ctx.enter_context(tc.tile_pool(name="x", bufs=1))
    wpool = ctx.enter_context(tc.tile_pool(name="w", bufs=1))
    qkv = ctx.enter_context(tc.tile_pool(name="qkv", bufs=1))

    # ---- persistent SBUF tiles ------------------------------------------
    x_sb = xpool.tile([128, 4 * T], BF16, name="xsb", tag="xsb")
    xk_sb = xpool.tile([128, 4 * NK], BF16, name="xksb", tag="xksb")
    ones_sb = xpool.tile([1, NK], BF16, name="ones", tag="ones")
    wq_sb = wpool.tile([128, 4 * 256], BF16, name="wq", tag="wq")
    wk_sb = wpool.tile([128, 4 * 256], BF16, name="wk", tag="wk")
    wv_sb = wpool.tile([128, 4 * 260], BF16, name="wv", tag="wv")
    wva_sb = wpool.tile([1, 260], BF16, name="wva", tag="wva")
    wo_sb = wpool.tile([128, 1024], BF16, name="wo", tag="wo")
    ebias = wpool.tile([128, 1], F32, name="eb", tag="eb")
    kT = [qkv.tile([128, NK], BF16, name=f"k{p}", tag=f"k{p}") for p in range(2)]
    qT = [qkv.tile([128, T], BF16, name=f"q{p}", tag=f"q{p}") for p in range(2)]
    v_all = qkv.tile([128, 4 * NKT * 65], BF16, name="v", tag="v")
    NDU = max(NKTR - NKTC, 1)
    kdup = qkv.tile([64, NDU * 128], BF16, name="kdup", tag="kdup")
    qdup = qkv.tile([64, T], BF16, name="qdup", tag="qdup")
    ctxn = [[qkv.tile([128, 512], BF16, name=f"cx{p}{qc}", tag=f"cx{p}{qc}")
             for qc in range(4)] for p in range(2)]

    def xv(sb, n):  # (ci, n) contraction-chunk view
        return sb[:].rearrange("p (ci n) -> p ci n", ci=4)

    x_v, xk_v = xv(x_sb, T), xv(xk_sb, NK)
    wq_v, wk_v = xv(wq_sb, 256), xv(wk_sb, 256)
    wv_v = xv(wv_sb, 260)
    v_v = v_all[:].rearrange("p (i kt d) -> p i kt d", i=4, kt=NKT)

    # ---- PSUM pools (all upfront: no cross-phase bank WAR surprises) ----
    # psS 4 banks + psX 2 + pp 2 = 8. pp serves projections AND out_proj.
    psS = ctx.enter_context(tc.tile_pool(name="psS", bufs=2, space="PSUM"))
    psX = ctx.enter_context(tc.tile_pool(name="psX", bufs=1, space="PSUM"))
    pp = ctx.enter_context(tc.tile_pool(name="pp", bufs=2, space="PSUM"))
    pbpool = ctx.enter_context(tc.tile_pool(name="pb", bufs=2))
    pN = ctx.enter_context(tc.tile_pool(name="pN", bufs=2))
    pO = ctx.enter_context(tc.tile_pool(name="pO", bufs=2))

    # ---- PE warmup: dummy matmuls during the initial DMA wait -----------
    warm = wpool.tile([128, 512], BF16, name="warm", tag="warm")
    nc.vector.memset(warm[:], 0.0)
    for _ in range(12):
        wp = pp.tile([128, 512], F32, name="wp", tag="pa")
        nc.tensor.matmul(wp[:], warm[:, 0:128], warm[:], start=True, stop=True)

    # ---- input DMAs: the first block's needs come first -----------------
    nc.sync.dma_start(wk_v[:, :, 0:128], wk_d.rearrange(
        "p (ci n) -> p ci n", ci=4)[:, :, 0:128])
    nc.sync.dma_start(wq_v[:, :, 0:128], wq_d.rearrange(
        "p (ci n) -> p ci n", ci=4)[:, :, 0:128])
    nkch = _chunks(NK, 512)
    xkd_v = xkT_d.rearrange("p (ci n) -> p ci n", ci=4)
    for n0, ns in nkch:
        nc.sync.dma_start(xk_v[:, :, n0:n0 + ns], xkd_v[:, :, n0:n0 + ns])
    nc.sync.dma_start(wv_sb[:], wv_d)
    nc.sync.dma_start(wva_sb[:], wva_d)
    nc.sync.dma_start(ones_sb[:], ones_d)
    nc.sync.dma_start(wk_v[:, :, 128:256], wk_d.rearrange(
        "p (ci n) -> p ci n", ci=4)[:, :, 128:256])
    nc.sync.dma_start(wq_v[:, :, 128:256], wq_d.rearrange(
        "p (ci n) -> p ci n", ci=4)[:, :, 128:256])
    xd_v = xT_d.rearrange("p (ci n) -> p ci n", ci=4)
    for qc in range(4):
        t0 = qc * 512
        nc.scalar.dma_start(x_v[:, :, t0:t0 + 512], xd_v[:, :, t0:t0 + 512])
    nc.sync.dma_start(wo_sb[:].rearrange("p (g c) -> p g c", g=2),
                      wo_d.rearrange("(g p) c -> p g c", p=128))
    if use_qbias:
        bq_sb = wpool.tile([128, 2], BF16, name="bq", tag="bq")
        nc.sync.dma_start(bq_sb[:], bq_d)
    nc.gpsimd.memset(ebias[:], float(EXP_SHIFT))

    # ---- projections (mostly deferred into block PE-slack) --------------
    def kq_proj(w_v, dst, pair, n0, ns, src_v):
        pt = pp.tile([128, 512], F32, name="pa", tag="pa")
        for ci in range(4):
            nc.tensor.matmul(
                pt[:, :ns], w_v[:, ci, pair * 128:(pair + 1) * 128],
                src_v[:, ci, n0:n0 + ns],
                start=(ci == 0), stop=(ci == 3))
        nc.vector.tensor_copy(dst[:, n0:n0 + ns], pt[:, :ns])

    def v_proj(kt):
        # regular-region ktiles only need heads 1-3; cdr ktiles head 0
        k0 = kt * 128
        c0, cn, i0, ni = (0, 65, 0, 1) if kt >= NKTR else (65, 195, 1, 3)
        pt = pp.tile([128, 512], F32, name="pv", tag="pa")
        for ci in range(4):
            nc.tensor.matmul(
                pt[:, 0:cn], xk_v[:, ci, k0:k0 + 128], wv_v[:, ci, c0:c0 + cn],
                start=(ci == 0), stop=False)
        nc.tensor.matmul(pt[:, 0:cn], ones_sb[:, k0:k0 + 128],
                         wva_sb[:, c0:c0 + cn], start=False, stop=True)
        nc.vector.tensor_copy(
            v_v[:, i0:i0 + ni, kt, :],
            pt[:, 0:cn].rearrange("p (i d) -> p i d", i=ni))

    for n0, ns in nkch[:2]:
        kq_proj(wk_v, kT[0], 0, n0, ns, xk_v)
    kq_proj(wq_v, qT[0], 0, 0, 512, x_v)  # first block's queries
    if NKTR > NKTC:
        # replica of head1's k rows in PE rows 0-63 lets the dual-slot
        # score matmuls run in both PE row groups concurrently
        nc.vector.tensor_copy(kdup[:], kT[0][64:128, 0:NDU * 128])
    bias_sb = None
    if use_qbias:
        # bias[key] = bq.k/8 + shift per (pair, head): tiny N=1 matmuls
        bias_sb = qkv.tile([128, 4 * NKT], F32, name="bias", tag="bias")

        def bias_mms(pair):
            for h in range(2):
                pt = pp.tile([128, 512], F32, name="pq", tag="pa")
                r0 = 64 * h
                for kt in range(NKT):
                    nc.tensor.matmul(
                        pt[:, kt:kt + 1],
                        kT[pair][r0:r0 + 64, kt * 128:(kt + 1) * 128],
                        bq_sb[r0:r0 + 64, pair:pair + 1],
                        start=True, stop=True)
                i = 2 * pair + h
                nc.vector.tensor_scalar_add(
                    bias_sb[:, i * NKT:(i + 1) * NKT], pt[:, 0:NKT],
                    EXP_SHIFT)
        bias_mms(0)

    # ---- attention blocks (qc outer, pair inner) ------------------------
    stage = {}

    def out_proj(qc, tq):
        cp = pp.tile([128, 512], F32, name="cp", tag="pa")
        for g in range(2):
            nc.tensor.matmul(cp[:], ctxn[g][qc][:, tq * 128:(tq + 1) * 128],
                             wo_sb[:, g * 512:(g + 1) * 512],
                             start=(g == 0), stop=(g == 1))
        if tq == 0:
            stage[qc] = pO.tile([128, 2048], BF16, name="o", tag="o")
        nc.vector.tensor_copy(stage[qc][:, tq * 512:(tq + 1) * 512], cp[:])
        r0 = qc * 512 + tq * 128
        nc.sync.dma_start(out_d[r0:r0 + 128, :],
                          stage[qc][:, tq * 512:(tq + 1) * 512])

    def normalize(pair, qc, h, ctx_ps_h):
        # cu copy frees the ctx PSUM bank; the reciprocal runs on a [64,8]
        # reshape (DVE reciprocal is ~6.5 cyc per free-elem, so keep the
        # free dim tiny), then a DRAM-row partition-broadcast DMA.
        idx = (qc * 4 + pair * 2 + h)
        cu = pN.tile([65, 512], F32, name=f"cu{h}", tag=f"cu{h}")
        nc.vector.tensor_copy(cu[:], ctx_ps_h[:])
        rs = pN.tile([64, 8], F32, name=f"rs{h}", tag=f"rs{h}")
        nc.gpsimd.dma_start(rs[:], cu[64:65, :])
        rr = pN.tile([64, 8], F32, name=f"rr{h}", tag=f"rr{h}")
        nc.vector.reciprocal(rr[:], rs[:])
        row = scr_d[idx:idx + 1, :]
        nc.gpsimd.dma_start(row.rearrange("a (p i) -> (a p) i", p=64), rr[:])
        bc = pN.tile([64, 512], F32, name=f"bc{h}", tag=f"bc{h}")
        nc.gpsimd.dma_start(bc[:], row.partition_broadcast(64))
        nc.vector.tensor_mul(
            ctxn[pair][qc][64 * h:64 * h + 64, :], cu[0:64, :], bc[:])

    def slot_plan(pair):
        # pair 0: head0 is the CDR head (cdr-region ktiles), head1 regular.
        # Leftover regular ktiles double up two-per-slot so exp ACTIVATEs
        # stay 1024 wide; those dual slots go FIRST so the earliest slots
        # only need the first xk DMA chunks.
        reg = list(range(NKTR))
        if pair == 1:
            return [[(0, k), (1, k)] for k in reg]
        cdr = list(range(NKTR, NKT))
        nd = NKTR - NKTC
        slots = []
        for a in range(0, nd, 2):
            ent = [(1, reg[a])]
            if a + 1 < nd:
                ent.append((1, reg[a + 1]))
            slots.append(ent)
        slots += [[(0, cdr[s]), (1, reg[nd + s])] for s in range(NKTC)]
        return slots

    projq = []  # urgent deferred projections: drained early in each block
    opq = []    # out_proj chunks: drained at alternating slots
    for qc in range(4):
        q0 = qc * 512
        for pair in range(2):
            bi = qc * 2 + pair
            if bi == 0:
                # kT[0] cdr-region chunks first (needed from slot 2), then
                # v tiles interleaved to match block-0 slot consumption
                # order (the drain schedule keeps each slot's v tiles
                # emitted ahead of the ctx matmuls that read them).
                projq += [(lambda a, b: lambda: kq_proj(
                    wk_v, kT[0], 0, a, b, xk_v))(n0_, ns_)
                    for n0_, ns_ in nkch[2:]]
                nd_ = NKTR - NKTC
                vorder = list(range(nd_))
                for s in range(NKTC):
                    vorder += [NKTR + s, nd_ + s]
                vps = [(lambda k: lambda: v_proj(k))(kt_) for kt_ in vorder]
                cut = max(len(vps) - 4, 0)
                projq += vps[:cut]
                projq.append(lambda: kq_proj(wq_v, qT[1], 1, 0, 512, x_v))
                projq += vps[cut:]
                projq.append(lambda: kq_proj(wk_v, kT[1], 1, *nkch[0], xk_v))
            elif bi == 1:
                projq += [(lambda a, b: lambda: kq_proj(
                    wk_v, kT[1], 1, a, b, xk_v))(n0_, ns_)
                    for n0_, ns_ in nkch[1:]]
                if use_qbias:
                    projq.append(lambda: bias_mms(1))
            if pair == 1 and qc < 3:
                projq += [(lambda pr, q: lambda: kq_proj(
                    wq_v, qT[pr], pr, q * 512, 512, x_v))(pr, qc + 1)
                    for pr in range(2)]
                if NKTR > NKTC:
                    projq.append((lambda q: lambda: nc.vector.tensor_copy(
                        qdup[0:64, q * 512:(q + 1) * 512],
                        qT[0][64:128, q * 512:(q + 1) * 512]))(qc + 1))
            if pair == 0 and qc == 0 and NKTR > NKTC:
                nc.vector.tensor_copy(qdup[0:64, q0:q0 + 512],
                                      qT[0][64:128, q0:q0 + 512])
            slots = slot_plan(pair)
            counts = [sum(1 for sl in slots for e in sl if e[0] == h)
                      for h in range(2)]
            pb = pbpool.tile([128, len(slots) * 1024], BF16,
                             name="pb", tag="pb")
            pb_v = pb[:].rearrange("p (sl x) -> p sl x", sl=len(slots))
            ctx_ps = [psX.tile([65, 512], F32, name=f"c{h}", tag=f"c{h}")
                      for h in range(2)]
            done = [0, 0]
            for si, ents in enumerate(slots):
                # one deferred-projection drain per slot keeps the PE burst
                # under the exp pace; block 0 needs higher throughput to
                # feed its own v tiles just-in-time.
                nd = 2 if bi == 0 else 1
                if si >= len(slots) - 2:
                    nd += 1
                if bi == 0 and si < 2:
                    nd += 2
                for _ in range(nd):
                    if projq:
                        projq.pop(0)()
                S = psS.tile([128, 1024], F32, name="S", tag="S")
                dual = len(ents) == 2 and ents[0][0] == ents[1][0]
                for e, (h, kt) in enumerate(ents):
                    if dual and e == 0:
                        nc.tensor.matmul(
                            S[:, 0:512],
                            kdup[0:64, kt * 128:(kt + 1) * 128],
                            qdup[0:64, q0:q0 + 512],
                            start=True, stop=True, tile_position=(0, 0))
                        continue
                    r0 = 64 * h
                    nc.tensor.matmul(
                        S[:, 512 * e:512 * e + 512],
                        kT[pair][r0:r0 + 64, kt * 128:(kt + 1) * 128],
                        qT[pair][r0:r0 + 64, q0:q0 + 512],
                        start=True, stop=True, tile_position=(r0, 0))
                # exp straight out of PSUM into the P buffer
                if use_qbias:
                    for e, (h, kt) in enumerate(ents):
                        i = 2 * pair + h
                        nc.scalar.activation(
                            pb_v[:, si, 512 * e:512 * e + 512],
                            S[:, 512 * e:512 * e + 512], Exp,
                            scale=EXP_SCALE,
                            bias=bias_sb[:, i * NKT + kt:i * NKT + kt + 1])
                else:
                    wid = 512 * len(ents)
                    nc.scalar.activation(
                        pb_v[:, si, 0:wid], S[:, 0:wid], Exp,
                        scale=EXP_SCALE, bias=ebias[:])
                # ctx accumulation (denominator rides in v column 64)
                for e, (h, kt) in enumerate(ents):
                    i = 2 * pair + h
                    nc.tensor.matmul(
                        ctx_ps[h][:], v_v[:, i, kt, :],
                        pb_v[:, si, 512 * e:512 * e + 512],
                        start=(done[h] == 0), stop=(done[h] == counts[h] - 1))
                    done[h] += 1
                if si % 2 == 0 and opq:
                    opq.pop(0)()
            for h in range(2):
                normalize(pair, qc, h, ctx_ps[h])
            if pair == 1:
                opq.extend(
                    (lambda q, t: lambda: out_proj(q, t))(qc, tq)
                    for tq in range(4))
    for fn in opq:
        fn()


# ---------------------------------------------------------------------------
# host side
# ---------------------------------------------------------------------------

def _round_up(n, m):
    return ((n + m - 1) // m) * m


def _ck(a):
    """[512, M] f32 -> [128, 4*M] bf16 with free layout (ci, m):
    contraction dim c = ci*128 + ki."""
    M = a.shape[1]
    out = np.ascontiguousarray(
        a.reshape(4, 128, M).transpose(1, 0, 2).reshape(128, 4 * M))
    return out.astype(ml_dtypes.bfloat16)


def _host_prep(x, mask, cdrs_score, Wq, bq, Wk, bk, Wv, bv, Wo, bo):
    x = np.asarray(x, np.float32)
    mask = np.asarray(mask)
    cdrs = np.asarray(cdrs_score)
    Wq = np.asarray(Wq, np.float32)
    Wk = np.asarray(Wk, np.float32)
    Wv = np.asarray(Wv, np.float32)
    Wo = np.asarray(Wo, np.float32)
    bq = np.asarray(bq, np.float32)
    bv = np.asarray(bv, np.float32)
    bo = np.asarray(bo, np.float32)

    gathers = []
    for b in range(B):
        valid = mask[b] == 1
        cdrv = valid & (cdrs[b] == 1) if np.any(cdrs[b] == 1) else valid
        gathers.append((np.nonzero(valid)[0], np.nonzero(cdrv)[0]))
    NKR = max(128, _round_up(max(len(g[0]) for g in gathers), 128))
    NCDR = max(128, _round_up(max(len(g[1]) for g in gathers), 128))
    NK = NKR + NCDR
    use_qbias = bool(np.any(bq != 0.0))

    # per-parity weight bundles (shared across samples)
    wbund = []
    bo_eff = []
    for p in range(2):
        heads = [p, p + 2, p + 4, p + 6]
        dims = np.concatenate([np.arange(h * D, (h + 1) * D) for h in heads])
        wq_ck = _ck(Wq[:, dims])
        wk_ck = _ck(Wk[:, dims])
        wv_cols = []
        for h in heads:
            hd = np.arange(h * D, (h + 1) * D)
            wv_cols.append(np.concatenate(
                [Wv[:, hd], np.zeros((C, 1), np.float32)], axis=1))
        wv_ck = _ck(np.concatenate(wv_cols, axis=1))
        wva = np.tile(np.concatenate(
            [np.zeros(64, np.float32), [1.0]]), 4)[None, :]
        wo_rows = np.ascontiguousarray(Wo[dims, :].astype(ml_dtypes.bfloat16))
        bq_arr = np.ascontiguousarray(
            (bq[dims] / 8.0).reshape(2, 128).T.astype(ml_dtypes.bfloat16))
        bo_eff.append(bv[dims] @ Wo[dims, :])
        wbund.append((wq_ck, wk_ck, wv_ck,
                      np.ascontiguousarray(wva.astype(ml_dtypes.bfloat16)),
                      wo_rows, bq_arr))
    bo_eff = bo + bo_eff[0] + bo_eff[1]

    in_maps = []
    for b in range(B):
        idx_all, idx_cdr = gathers[b]
        xk = np.zeros((NK, C), np.float32)
        ones_row = np.zeros((1, NK), np.float32)
        xk[:len(idx_all)] = x[b, idx_all]
        ones_row[0, :len(idx_all)] = 1.0
        xk[NKR:NKR + len(idx_cdr)] = x[b, idx_cdr]
        ones_row[0, NKR:NKR + len(idx_cdr)] = 1.0
        xT_ck = _ck(np.ascontiguousarray(x[b].T).reshape(C, T))
        xkT_ck = _ck(np.ascontiguousarray(xk.T))
        ones_bf = np.ascontiguousarray(ones_row.astype(ml_dtypes.bfloat16))
        for p in range(2):
            wq_ck, wk_ck, wv_ck, wva, wo_rows, bq_arr = wbund[p]
            im = {"xT": xT_ck, "xkT": xkT_ck, "ones": ones_bf,
                  "Wq": wq_ck, "Wk": wk_ck, "Wv": wv_ck, "Wva": wva,
                  "Wo": wo_rows}
            if use_qbias:
                im["bq"] = bq_arr
            in_maps.append(im)
    return in_maps, NKR, NCDR, use_qbias, bo_eff


def kernel(**inputs) -> np.ndarray:
    global LAST_RESULTS
    in_maps, NKR, NCDR, use_qbias, bo_eff = _host_prep(**inputs)

    key = (NKR, NCDR, use_qbias)
    nc = _PROGRAM_CACHE.get(key)
    if nc is None:
        nc = _build_program(NKR, NCDR, use_qbias)
        _PROGRAM_CACHE[key] = nc

    res = run_bass_kernel_spmd(nc, in_maps, core_ids=list(range(8)))
    LAST_RESULTS = res

    out = np.empty((B, T, C), np.float32)
    for b in range(B):
        out[b] = (np.asarray(res.results[2 * b]["out"], np.float32)
                  + np.asarray(res.results[2 * b + 1]["out"], np.float32)
                  + bo_eff[None, :])
    return out


# revision 37
# speedup vs baseline: 1.0772x; 1.0772x over previous
"""Trainium2 Bass kernel for nn_CDRsAttention (sparse multi-head attention
with padding mask + CDR key mask on the first 2 heads).

Sharding: 8 cores = 4 samples (B) x 2 head-parity groups. Core (b, p)
computes heads [p, p+2, p+4, p+6] of sample b (exactly one CDR head each,
so all cores do identical work), producing a partial output
ctx_heads @ Wo_rows; the host sums the two parity partials + bo_eff.

Host-side prep (pure numpy, cheap):
  - two key gathers per sample: region R = all keys with mask==1 (regular
    heads attend these), region C = the CDR-valid subset again (the CDR
    head attends only these). Separate regions cost duplicate k/v
    projection work but minimize padded exp slots, which is what the
    ScalarE exp roofline is made of. Pad slots have k=v=0 and a 0 in the
    ones-row, so they drop out of softmax numerator and denominator.
  - bias algebra: bk is softmax-invariant (dropped), bv shifts ctx by a
    constant per head (folded into bo on host), bq contributes bq.k per
    key (applied through the exp bias operand only when nonzero; zero
    here). Only the v projection keeps an augmented ones-row, feeding a
    selector column so ctx row 64 is the softmax denominator.

Device per core (bf16 data, f32 accumulation):
  warmup matmuls spin the PE clock up during the initial DMAs; k/q/v
  projections are emitted just-in-time inside the attention blocks' PE
  slack. Per (query-chunk, head-pair) block: packed score matmuls (two
  64-row PE groups) fill S [128,1024] slots pairing the CDR head's
  ktiles with regular ktiles (and doubling up leftover regular ktiles),
  so nearly every exp ACTIVATE on ScalarE - the binding engine at
  T x sum_h(keys_h) elements, 1/lane/cycle - is full width. ctx^T
  accumulates in PSUM with the denominator in row 64; normalize =
  DVE copy (frees PSUM) + reciprocal on a [64,8] reshape (DVE reciprocal
  is ~6.5 cyc/free-elem, so keep the free dim tiny) + a DRAM-row
  partition-broadcast DMA + DVE multiply. Output projection chunks drain
  into remaining PE slack; per-128-row bf16 output DMAs overlap the
  tail.
"""
import os
from contextlib import ExitStack

import ml_dtypes
import numpy as np

import concourse.bass as bass
import concourse.mybir as mybir
import concourse.tile as tile
from concourse import bacc
from concourse.bass_utils import run_bass_kernel_spmd

B, T, C, H, D = 4, 2048, 512, 8, 64
F32 = mybir.dt.float32
BF16 = mybir.dt.bfloat16
EXP_SHIFT = -3.0  # exp(s - 3): softmax-invariant (keeps P modest)
EXP_SCALE = 1.0 / 8.0

_PROGRAM_CACHE: dict = {}
LAST_RESULTS = None  # BassKernelResults of the most recent kernel() call


def _chunks(total, step):
    return [(i, min(step, total - i)) for i in range(0, total, step)]


def _build_program(NKR, NCDR, use_qbias):
    NK = NKR + NCDR  # total key slots: [regular | cdr]
    NKT = NK // 128
    NKTR = NKR // 128
    NKTC = NCDR // 128

    nc = bacc.Bacc("TRN2", target_bir_lowering=False, debug=False, num_devices=8)
    # free layouts: xT/xkT/wq/wk/wv are (ci, n): contraction c = ci*128+ki
    xT_d = nc.dram_tensor("xT", [128, 4 * T], BF16, kind="ExternalInput").ap()
    xkT_d = nc.dram_tensor("xkT", [128, 4 * NK], BF16, kind="ExternalInput").ap()
    ones_d = nc.dram_tensor("ones", [1, NK], BF16, kind="ExternalInput").ap()
    wq_d = nc.dram_tensor("Wq", [128, 4 * 256], BF16, kind="ExternalInput").ap()
    wk_d = nc.dram_tensor("Wk", [128, 4 * 256], BF16, kind="ExternalInput").ap()
    wv_d = nc.dram_tensor("Wv", [128, 4 * 260], BF16, kind="ExternalInput").ap()
    wva_d = nc.dram_tensor("Wva", [1, 260], BF16, kind="ExternalInput").ap()
    wo_d = nc.dram_tensor("Wo", [256, 512], BF16, kind="ExternalInput").ap()
    bq_d = (nc.dram_tensor("bq", [128, 2], BF16, kind="ExternalInput").ap()
            if use_qbias else None)
    out_d = nc.dram_tensor("out", [T, 512], BF16, kind="ExternalOutput").ap()
    scr_d = nc.dram_tensor("scr", [16, 512], F32, kind="Internal").ap()

    with tile.TileContext(nc) as tc:
        with ExitStack() as ctx:
            _body(ctx, tc, xT_d, xkT_d, ones_d, wq_d, wk_d, wv_d, wva_d,
                  wo_d, bq_d, out_d, scr_d, NK, NKT, NKTR, NKTC, use_qbias)
    nc.compile()
    return nc


def _body(ctx, tc, xT_d, xkT_d, ones_d, wq_d, wk_d, wv_d, wva_d, wo_d, bq_d,
          out_d, scr_d, NK, NKT, NKTR, NKTC, use_qbias):
    nc = tc.nc
    Exp = mybir.ActivationFunctionType.Exp

    xpool = ctx.enter_context(tc.tile_pool(name="x", bufs=1))
    wpool = ctx.enter_context(tc.tile_pool(name="w", bufs=1))
    qkv = ctx.enter_context(tc.tile_pool(name="qkv", bufs=1))

    # ---- persistent SBUF tiles ------------------------------------------
    x_sb = xpool.tile([128, 4 * T], BF16, name="xsb", tag="xsb")
    xk_sb = xpool.tile([128, 4 * NK], BF16, name="xksb", tag="xksb")
    ones_sb = xpool.tile([1, NK], BF16, name="ones", tag="ones")
    wq_sb = wpool.tile([128, 4 * 256], BF16, name="wq", tag="wq")
    wk_sb = wpool.tile([128, 4 * 256], BF16, name="wk", tag="wk")
    wv_sb = wpool.tile([128, 4 * 260], BF16, name="wv", tag="wv")
    wva_sb = wpool.tile([1, 260], BF16, name="wva", tag="wva")
    wo_sb = wpool.tile([128, 1024], BF16, name="wo", tag="wo")
    ebias = wpool.tile([128, 1], F32, name="eb", tag="eb")
    kT = [qkv.tile([128, NK], BF16, name=f"k{p}", tag=f"k{p}") for p in range(2)]
    qT = [qkv.tile([128, T], BF16, name=f"q{p}", tag=f"q{p}") for p in range(2)]
    v_all = qkv.tile([128, 4 * NKT * 65], BF16, name="v", tag="v")
    NDU = max(NKTR - NKTC, 1)
    kdup = qkv.tile([64, NDU * 128], BF16, name="kdup", tag="kdup")
    qdup = qkv.tile([64, T], BF16, name="qdup", tag="qdup")
    ctxn = [[qkv.tile([128, 512], BF16, name=f"cx{p}{qc}", tag=f"cx{p}{qc}")
             for qc in range(4)] for p in range(2)]

    def xv(sb, n):  # (ci, n) contraction-chunk view
        return sb[:].rearrange("p (ci n) -> p ci n", ci=4)

    x_v, xk_v = xv(x_sb, T), xv(xk_sb, NK)
    wq_v, wk_v = xv(wq_sb, 256), xv(wk_sb, 256)
    wv_v = xv(wv_sb, 260)
    v_v = v_all[:].rearrange("p (i kt d) -> p i kt d", i=4, kt=NKT)

    # ---- PSUM pools (all upfront: no cross-phase bank WAR surprises) ----
    # psS 4 banks + psX 2 + pp 2 = 8. pp serves projections AND out_proj.
    psS = ctx.enter_context(tc.tile_pool(name="psS", bufs=2, space="PSUM"))
    psX = ctx.enter_context(tc.tile_pool(name="psX", bufs=1, space="PSUM"))
    pp = ctx.enter_context(tc.tile_pool(name="pp", bufs=2, space="PSUM"))
    pbpool = ctx.enter_context(tc.tile_pool(name="pb", bufs=2))
    pN = ctx.enter_context(tc.tile_pool(name="pN", bufs=2))
    pO = ctx.enter_context(tc.tile_pool(name="pO", bufs=2))

    # ---- PE warmup: dummy matmuls during the initial DMA wait -----------
    warm = wpool.tile([128, 512], BF16, name="warm", tag="warm")
    nc.vector.memset(warm[:], 0.0)
    for _ in range(12):
        wp = pp.tile([128, 512], F32, name="wp", tag="pa")
        nc.tensor.matmul(wp[:], warm[:, 0:128], warm[:], start=True, stop=True)

    # ---- input DMAs: the first block's needs come first -----------------
    nc.sync.dma_start(wk_v[:, :, 0:128], wk_d.rearrange(
        "p (ci n) -> p ci n", ci=4)[:, :, 0:128])
    nc.sync.dma_start(wq_v[:, :, 0:128], wq_d.rearrange(
        "p (ci n) -> p ci n", ci=4)[:, :, 0:128])
    nkch = _chunks(NK, 512)
    xkd_v = xkT_d.rearrange("p (ci n) -> p ci n", ci=4)
    for n0, ns in nkch:
        nc.sync.dma_start(xk_v[:, :, n0:n0 + ns], xkd_v[:, :, n0:n0 + ns])
    nc.sync.dma_start(wv_sb[:], wv_d)
    nc.sync.dma_start(wva_sb[:], wva_d)
    nc.sync.dma_start(ones_sb[:], ones_d)
    nc.sync.dma_start(wk_v[:, :, 128:256], wk_d.rearrange(
        "p (ci n) -> p ci n", ci=4)[:, :, 128:256])
    nc.sync.dma_start(wq_v[:, :, 128:256], wq_d.rearrange(
        "p (ci n) -> p ci n", ci=4)[:, :, 128:256])
    xd_v = xT_d.rearrange("p (ci n) -> p ci n", ci=4)
    for qc in range(4):
        t0 = qc * 512
        nc.scalar.dma_start(x_v[:, :, t0:t0 + 512], xd_v[:, :, t0:t0 + 512])
    nc.sync.dma_start(wo_sb[:].rearrange("p (g c) -> p g c", g=2),
                      wo_d.rearrange("(g p) c -> p g c", p=128))
    if use_qbias:
        bq_sb = wpool.tile([128, 2], BF16, name="bq", tag="bq")
        nc.sync.dma_start(bq_sb[:], bq_d)
    nc.gpsimd.memset(ebias[:], float(EXP_SHIFT))

    # ---- projections (mostly deferred into block PE-slack) --------------
    def kq_proj(w_v, dst, pair, n0, ns, src_v):
        pt = pp.tile([128, 512], F32, name="pa", tag="pa")
        for ci in range(4):
            nc.tensor.matmul(
                pt[:, :ns], w_v[:, ci, pair * 128:(pair + 1) * 128],
                src_v[:, ci, n0:n0 + ns],
                start=(ci == 0), stop=(ci == 3))
        nc.vector.tensor_copy(dst[:, n0:n0 + ns], pt[:, :ns])

    def v_proj(kt):
        # regular-region ktiles only need heads 1-3; cdr ktiles head 0
        k0 = kt * 128
        c0, cn, i0, ni = (0, 65, 0, 1) if kt >= NKTR else (65, 195, 1, 3)
        pt = pp.tile([128, 512], F32, name="pv", tag="pa")
        for ci in range(4):
            nc.tensor.matmul(
                pt[:, 0:cn], xk_v[:, ci, k0:k0 + 128], wv_v[:, ci, c0:c0 + cn],
                start=(ci == 0), stop=False)
        nc.tensor.matmul(pt[:, 0:cn], ones_sb[:, k0:k0 + 128],
                         wva_sb[:, c0:c0 + cn], start=False, stop=True)
        nc.vector.tensor_copy(
            v_v[:, i0:i0 + ni, kt, :],
            pt[:, 0:cn].rearrange("p (i d) -> p i d", i=ni))

    for n0, ns in nkch[:2]:
        kq_proj(wk_v, kT[0], 0, n0, ns, xk_v)
    kq_proj(wq_v, qT[0], 0, 0, 512, x_v)  # first block's queries
    if NKTR > NKTC:
        # replica of head1's k rows in PE rows 0-63 lets the dual-slot
        # score matmuls run in both PE row groups concurrently
        nc.vector.tensor_copy(kdup[:], kT[0][64:128, 0:NDU * 128])
    bias_sb = None
    if use_qbias:
        # bias[key] = bq.k/8 + shift per (pair, head): tiny N=1 matmuls
        bias_sb = qkv.tile([128, 4 * NKT], F32, name="bias", tag="bias")

        def bias_mms(pair):
            for h in range(2):
                pt = pp.tile([128, 512], F32, name="pq", tag="pa")
                r0 = 64 * h
                for kt in range(NKT):
                    nc.tensor.matmul(
                        pt[:, kt:kt + 1],
                        kT[pair][r0:r0 + 64, kt * 128:(kt + 1) * 128],
                        bq_sb[r0:r0 + 64, pair:pair + 1],
                        start=True, stop=True)
                i = 2 * pair + h
                nc.vector.tensor_scalar_add(
                    bias_sb[:, i * NKT:(i + 1) * NKT], pt[:, 0:NKT],
                    EXP_SHIFT)
        bias_mms(0)

    # ---- attention blocks (qc outer, pair inner) ------------------------
    stage = {}

    def out_proj(qc, tq):
        cp = pp.tile([128, 512], F32, name="cp", tag="pa")
        for g in range(2):
            nc.tensor.matmul(cp[:], ctxn[g][qc][:, tq * 128:(tq + 1) * 128],
                             wo_sb[:, g * 512:(g + 1) * 512],
                             start=(g == 0), stop=(g == 1))
        if tq == 0:
            stage[qc] = pO.tile([128, 2048], BF16, name="o", tag="o")
        nc.vector.tensor_copy(stage[qc][:, tq * 512:(tq + 1) * 512], cp[:])
        r0 = qc * 512 + tq * 128
        nc.sync.dma_start(out_d[r0:r0 + 128, :],
                          stage[qc][:, tq * 512:(tq + 1) * 512])

    def normalize(pair, qc, h, ctx_ps_h):
        # cu copy frees the ctx PSUM bank; the reciprocal runs on a [64,8]
        # reshape (DVE reciprocal is ~6.5 cyc per free-elem, so keep the
        # free dim tiny), then a DRAM-row partition-broadcast DMA.
        idx = (qc * 4 + pair * 2 + h)
        cu = pN.tile([65, 512], F32, name=f"cu{h}", tag=f"cu{h}")
        nc.vector.tensor_copy(cu[:], ctx_ps_h[:])
        rs = pN.tile([64, 8], F32, name=f"rs{h}", tag=f"rs{h}")
        nc.gpsimd.dma_start(rs[:], cu[64:65, :])
        rr = pN.tile([64, 8], F32, name=f"rr{h}", tag=f"rr{h}")
        nc.vector.reciprocal(rr[:], rs[:])
        row = scr_d[idx:idx + 1, :]
        nc.gpsimd.dma_start(row.rearrange("a (p i) -> (a p) i", p=64), rr[:])
        bc = pN.tile([64, 512], F32, name=f"bc{h}", tag=f"bc{h}")
        nc.gpsimd.dma_start(bc[:], row.partition_broadcast(64))
        nc.vector.tensor_mul(
            ctxn[pair][qc][64 * h:64 * h + 64, :], cu[0:64, :], bc[:])

    def slot_plan(pair):
        # pair 0: head0 is the CDR head (cdr-region ktiles), head1 regular.
        # Leftover regular ktiles double up two-per-slot so exp ACTIVATEs
        # stay 1024 wide; those dual slots go FIRST so the earliest slots
        # only need the first xk DMA chunks.
        reg = list(range(NKTR))
        if pair == 1:
            return [[(0, k), (1, k)] for k in reg]
        cdr = list(range(NKTR, NKT))
        nd = NKTR - NKTC
        slots = []
        for a in range(0, nd, 2):
            ent = [(1, reg[a])]
            if a + 1 < nd:
                ent.append((1, reg[a + 1]))
            slots.append(ent)
        slots += [[(0, cdr[s]), (1, reg[nd + s])] for s in range(NKTC)]
        return slots

    projq = []  # urgent deferred projections: drained early in each block
    opq = []    # out_proj chunks: drained at alternating slots
    for qc in range(4):
        q0 = qc * 512
        for pair in range(2):
            bi = qc * 2 + pair
            if bi == 0:
                # kT[0] cdr-region chunks first (needed from slot 2), then
                # v tiles interleaved to match block-0 slot consumption
                # order (the drain schedule keeps each slot's v tiles
                # emitted ahead of the ctx matmuls that read them).
                projq += [(lambda a, b: lambda: kq_proj(
                    wk_v, kT[0], 0, a, b, xk_v))(n0_, ns_)
                    for n0_, ns_ in nkch[2:]]
                nd_ = NKTR - NKTC
                vorder = list(range(nd_))
                for s in range(NKTC):
                    vorder += [NKTR + s, nd_ + s]
                vps = [(lambda k: lambda: v_proj(k))(kt_) for kt_ in vorder]
                cut = max(len(vps) - 4, 0)
                projq += vps[:cut]
                projq.append(lambda: kq_proj(wq_v, qT[1], 1, 0, 512, x_v))
                projq += vps[cut:]
                nkch1 = _chunks(NKTR * 128, 512)
                projq.append(lambda: kq_proj(wk_v, kT[1], 1, *nkch1[0], xk_v))
            elif bi == 1:
                nkch1 = _chunks(NKTR * 128, 512)
                projq += [(lambda a, b: lambda: kq_proj(
                    wk_v, kT[1], 1, a, b, xk_v))(n0_, ns_)
                    for n0_, ns_ in nkch1[1:]]
                if use_qbias:
                    projq.append(lambda: bias_mms(1))
            if pair == 1 and qc < 3:
                projq += [(lambda pr, q: lambda: kq_proj(
                    wq_v, qT[pr], pr, q * 512, 512, x_v))(pr, qc + 1)
                    for pr in range(2)]
                if NKTR > NKTC:
                    projq.append((lambda q: lambda: nc.vector.tensor_copy(
                        qdup[0:64, q * 512:(q + 1) * 512],
                        qT[0][64:128, q * 512:(q + 1) * 512]))(qc + 1))
            if pair == 0 and qc == 0 and NKTR > NKTC:
                nc.vector.tensor_copy(qdup[0:64, q0:q0 + 512],
                                      qT[0][64:128, q0:q0 + 512])
            slots = slot_plan(pair)
            counts = [sum(1 for sl in slots for e in sl if e[0] == h)
                      for h in range(2)]
            pb = pbpool.tile([128, len(slots) * 1024], BF16,
                             name="pb", tag="pb")
            pb_v = pb[:].rearrange("p (sl x) -> p sl x", sl=len(slots))
            ctx_ps = [psX.tile([65, 512], F32, name=f"c{h}", tag=f"c{h}")
                      for h in range(2)]
            done = [0, 0]
            for si, ents in enumerate(slots):
                nd = 2 + (1 if si >= len(slots) - 2 else 0)
                if bi == 0 and si < 2:
                    nd += 2
                for _ in range(nd):
                    if projq:
                        projq.pop(0)()
                S = psS.tile([128, 1024], F32, name="S", tag="S")
                dual = len(ents) == 2 and ents[0][0] == ents[1][0]
                for e, (h, kt) in enumerate(ents):
                    if dual and e == 0:
                        nc.tensor.matmul(
                            S[:, 0:512],
                            kdup[0:64, kt * 128:(kt + 1) * 128],
                            qdup[0:64, q0:q0 + 512],
                            start=True, stop=True, tile_position=(0, 0))
                        continue
                    r0 = 64 * h
                    nc.tensor.matmul(
                        S[:, 512 * e:512 * e + 512],
                        kT[pair][r0:r0 + 64, kt * 128:(kt + 1) * 128],
                        qT[pair][r0:r0 + 64, q0:q0 + 512],
                        start=True, stop=True, tile_position=(r0, 0))
                # exp straight out of PSUM into the P buffer
                if use_qbias:
                    for e, (h, kt) in enumerate(ents):
                        i = 2 * pair + h
                        nc.scalar.activation(
                            pb_v[:, si, 512 * e:512 * e + 512],
                            S[:, 512 * e:512 * e + 512], Exp,
                            scale=EXP_SCALE,
                            bias=bias_sb[:, i * NKT + kt:i * NKT + kt + 1])
                else:
                    wid = 512 * len(ents)
                    nc.scalar.activation(
                        pb_v[:, si, 0:wid], S[:, 0:wid], Exp,
                        scale=EXP_SCALE, bias=ebias[:])
                # ctx accumulation (denominator rides in v column 64)
                for e, (h, kt) in enumerate(ents):
                    i = 2 * pair + h
                    nc.tensor.matmul(
                        ctx_ps[h][:], v_v[:, i, kt, :],
                        pb_v[:, si, 512 * e:512 * e + 512],
                        start=(done[h] == 0), stop=(done[h] == counts[h] - 1))
                    done[h] += 1
                if si % 2 == 0 and opq:
                    opq.pop(0)()
            for h in range(2):
                normalize(pair, qc, h, ctx_ps[h])
            if pair == 1:
                opq.extend(
                    (lambda q, t: lambda: out_proj(q, t))(qc, tq)
                    for tq in range(4))
    for fn in opq:
        fn()


# ---------------------------------------------------------------------------
# host side
# ---------------------------------------------------------------------------

def _round_up(n, m):
    return ((n + m - 1) // m) * m


def _ck(a):
    """[512, M] f32 -> [128, 4*M] bf16 with free layout (ci, m):
    contraction dim c = ci*128 + ki."""
    M = a.shape[1]
    out = np.ascontiguousarray(
        a.reshape(4, 128, M).transpose(1, 0, 2).reshape(128, 4 * M))
    return out.astype(ml_dtypes.bfloat16)


def _host_prep(x, mask, cdrs_score, Wq, bq, Wk, bk, Wv, bv, Wo, bo):
    x = np.asarray(x, np.float32)
    mask = np.asarray(mask)
    cdrs = np.asarray(cdrs_score)
    Wq = np.asarray(Wq, np.float32)
    Wk = np.asarray(Wk, np.float32)
    Wv = np.asarray(Wv, np.float32)
    Wo = np.asarray(Wo, np.float32)
    bq = np.asarray(bq, np.float32)
    bv = np.asarray(bv, np.float32)
    bo = np.asarray(bo, np.float32)

    gathers = []
    for b in range(B):
        valid = mask[b] == 1
        cdrv = valid & (cdrs[b] == 1) if np.any(cdrs[b] == 1) else valid
        gathers.append((np.nonzero(valid)[0], np.nonzero(cdrv)[0]))
    NKR = max(128, _round_up(max(len(g[0]) for g in gathers), 128))
    NCDR = max(128, _round_up(max(len(g[1]) for g in gathers), 128))
    NK = NKR + NCDR
    use_qbias = bool(np.any(bq != 0.0))

    # per-parity weight bundles (shared across samples)
    wbund = []
    bo_eff = []
    for p in range(2):
        heads = [p, p + 2, p + 4, p + 6]
        dims = np.concatenate([np.arange(h * D, (h + 1) * D) for h in heads])
        wq_ck = _ck(Wq[:, dims])
        wk_ck = _ck(Wk[:, dims])
        wv_cols = []
        for h in heads:
            hd = np.arange(h * D, (h + 1) * D)
            wv_cols.append(np.concatenate(
                [Wv[:, hd], np.zeros((C, 1), np.float32)], axis=1))
        wv_ck = _ck(np.concatenate(wv_cols, axis=1))
        wva = np.tile(np.concatenate(
            [np.zeros(64, np.float32), [1.0]]), 4)[None, :]
        wo_rows = np.ascontiguousarray(Wo[dims, :].astype(ml_dtypes.bfloat16))
        bq_arr = np.ascontiguousarray(
            (bq[dims] / 8.0).reshape(2, 128).T.astype(ml_dtypes.bfloat16))
        bo_eff.append(bv[dims] @ Wo[dims, :])
        wbund.append((wq_ck, wk_ck, wv_ck,
                      np.ascontiguousarray(wva.astype(ml_dtypes.bfloat16)),
                      wo_rows, bq_arr))
    bo_eff = bo + bo_eff[0] + bo_eff[1]

    in_maps = []
    for b in range(B):
        idx_all, idx_cdr = gathers[b]
        xk = np.zeros((NK, C), np.float32)
        ones_row = np.zeros((1, NK), np.float32)
        xk[:len(idx_all)] = x[b, idx_all]
        ones_row[0, :len(idx_all)] = 1.0
        xk[NKR:NKR + len(idx_cdr)] = x[b, idx_cdr]
        ones_row[0, NKR:NKR + len(idx_cdr)] = 1.0
        xT_ck = _ck(np.ascontiguousarray(x[b].T).reshape(C, T))
        xkT_ck = _ck(np.ascontiguousarray(xk.T))
        ones_bf = np.ascontiguousarray(ones_row.astype(ml_dtypes.bfloat16))
        for p in range(2):
            wq_ck, wk_ck, wv_ck, wva, wo_rows, bq_arr = wbund[p]
            im = {"xT": xT_ck, "xkT": xkT_ck, "ones": ones_bf,
                  "Wq": wq_ck, "Wk": wk_ck, "Wv": wv_ck, "Wva": wva,
                  "Wo": wo_rows}
            if use_qbias:
                im["bq"] = bq_arr
            in_maps.append(im)
    return in_maps, NKR, NCDR, use_qbias, bo_eff


def kernel(**inputs) -> np.ndarray:
    global LAST_RESULTS
    in_maps, NKR, NCDR, use_qbias, bo_eff = _host_prep(**inputs)

    key = (NKR, NCDR, use_qbias)
    nc = _PROGRAM_CACHE.get(key)
    if nc is None:
        nc = _build_program(NKR, NCDR, use_qbias)
        _PROGRAM_CACHE[key] = nc

    res = run_bass_kernel_spmd(nc, in_maps, core_ids=list(range(8)))
    LAST_RESULTS = res

    out = np.empty((B, T, C), np.float32)
    for b in range(B):
        out[b] = (np.asarray(res.results[2 * b]["out"], np.float32)
                  + np.asarray(res.results[2 * b + 1]["out"], np.float32)
                  + bo_eff[None, :])
    return out
